# revision 8
# baseline (speedup 1.0000x reference)
"""DGDAGRNN (DAG-GRU message passing) Trainium2 kernel, 8 NeuronCores.

Strategy
--------
Nodes are globally re-permuted host-side, sorted by (topological layer,
in-degree desc) and dealt round-robin to 8 devices so each (layer, device)
block is a contiguous, degree-sorted, 128-padded row range of a replicated
hidden-state table  h_dram [N_pad+pad, 128]  (feat dim padded 100->128).

Per step (round r, layer l), on every device:
  1. indirect-DMA gather of source-node h rows for the device's layer-l
     in-edges.  Host pre-arranges edge slots into "levels" (k-th in-edge of
     each destination) so the segment-sum becomes a handful of contiguous
     vector adds; missing slots point at a dedicated zero row (gn(0) == 0
     since the mapper matmul has no bias).  In round 0, sources that have
     not been updated yet are also redirected to the zero row (h==0 then).
  2. PE-transpose gathered row tiles to feature-major, compute the gated
     message gn = sigmoid(Wg h + bg) * (Wm h) per edge slot with
     weight-stationary matmuls.
  3. Level-wise contiguous adds produce msgT for the device's m_l block.
  4. GRU cell (precomputed input-side gates gi from x), classifier head on
     the last round, transpose h_new back to row-major.
  5. AllGather the 8 devices' [m_l, 128] blocks directly into the step's
     region of every replica's h_dram.

Self-contained: only numpy + the concourse (Bass/Tile) stack.
"""

import numpy as np

NDEV = 8
P = 128
FW = 128          # padded feature width of h rows (VHS=100 used)
MMCHUNK = 512     # matmul free-dim chunk (one PSUM bank of fp32)
GATHER_COLS = 8   # index columns (x128 rows) per indirect DMA


# --------------------------------------------------------------------------
# host-side preprocessing
# --------------------------------------------------------------------------

def preprocess(node_layer, edge_src, edge_dst, L):
    node_layer = np.asarray(node_layer).astype(np.int64)
    edge_src = np.asarray(edge_src).astype(np.int64)
    edge_dst = np.asarray(edge_dst).astype(np.int64)
    N = node_layer.shape[0]
    E = edge_src.shape[0]
    deg = np.bincount(edge_dst, minlength=N).astype(np.int64)

    # node permutation: (layer, deg desc, id), dealt round-robin to devices
    order = np.lexsort((np.arange(N), -deg, node_layer))
    cnt_l = np.bincount(node_layer, minlength=L).astype(np.int64)
    m = ((-(-cnt_l // NDEV) + P - 1) // P) * P           # block rows per (l, d)
    block_start = np.zeros(L + 1, np.int64)
    block_start[1:] = np.cumsum(NDEV * m)
    cl0 = np.zeros(L, np.int64)
    cl0[1:] = np.cumsum(cnt_l)[:-1]
    lay_s = node_layer[order]
    j_in_layer = np.arange(N) - cl0[lay_s]
    newpos = np.empty(N, np.int64)
    newpos[order] = block_start[lay_s] + (j_in_layer % NDEV) * m[lay_s] \
        + j_in_layer // NDEV
    N_pad = int(block_start[L])
    zero_row = N_pad

    # edges -> (layer, device, local idx, occurrence k among same dst)
    v = edge_dst
    le = node_layer[v]
    pe = newpos[v]
    rel = pe - block_start[le]
    de = rel // m[le]
    ie = rel % m[le]
    eorder = np.lexsort((np.arange(E), pe))
    pe_s = pe[eorder]
    newgrp = np.r_[True, pe_s[1:] != pe_s[:-1]]
    starts = np.flatnonzero(newgrp)
    grp = np.cumsum(newgrp) - 1
    ke = np.empty(E, np.int64)
    ke[eorder] = np.arange(E) - starts[grp]
    maxdeg = int(ke.max()) + 1

    # level sizes, uniform across devices; level 0 covers the whole block
    tmp = np.zeros(L * maxdeg * NDEV, np.int64)
    key = (le * maxdeg + ke) * NDEV + de
    np.maximum.at(tmp, key, ie + 1)
    n_lk = tmp.reshape(L, maxdeg, NDEV).max(axis=2)
    n_lk[:, 0] = m
    lvl_off = np.zeros((L, maxdeg + 1), np.int64)
    lvl_off[:, 1:] = np.cumsum(n_lk, axis=1)
    tot_l = lvl_off[:, -1]
    E_pad_l = ((tot_l + P - 1) // P) * P
    step_off = np.zeros(L + 1, np.int64)
    step_off[1:] = np.cumsum(E_pad_l)
    TOT = int(step_off[L])

    # flat per-device source-index arrays (kind 0: round-0 masked, kind 1: rest)
    src_p = newpos[edge_src]
    slot = step_off[le] + lvl_off[le, ke] + ie
    src1 = np.full((NDEV, TOT), zero_row, np.int64)
    src1[de, slot] = src_p
    src0 = np.full((NDEV, TOT), zero_row, np.int64)
    src0[de, slot] = np.where(node_layer[edge_src] < le, src_p, zero_row)

    return dict(order=order, cnt_l=cnt_l, cl0=cl0, newpos=newpos, m=m,
                block_start=block_start, n_lk=n_lk, lvl_off=lvl_off,
                maxdeg=maxdeg, src0=src0, src1=src1, N_pad=N_pad,
                zero_row=zero_row, E_pad_l=E_pad_l, step_off=step_off, TOT=TOT)


def build_core_inputs(pp, inputs, L):
    """Per-core input dicts (weights replicated, indices/x per device)."""
    x = np.asarray(inputs["x"], np.float32)
    m = pp["m"]
    M_tot = int(m.sum())
    own_off = np.zeros(L + 1, np.int64)
    own_off[1:] = np.cumsum(m)
    TOT = pp["TOT"]

    # index tiles: [128, 2*TOT/128], column block for (kind, l); within a step
    # idx[p, j] = src_flat[j*128 + p]
    def idx_tile(src_flat):
        return np.ascontiguousarray(
            src_flat.reshape(TOT // P, P).T).astype(np.int32)

    W_ih = np.asarray(inputs["W_ih"], np.float32)
    W_hh = np.asarray(inputs["W_hh"], np.float32)
    W_g = np.asarray(inputs["W_g"], np.float32)
    W_m = np.asarray(inputs["W_m"], np.float32)
    W_c1 = np.asarray(inputs["W_c1"], np.float32)
    W_c2 = np.asarray(inputs["W_c2"], np.float32)
    VHS = W_g.shape[0]
    CHS = W_c1.shape[0]
    common = {
        "WgT": np.ascontiguousarray(W_g.T),
        "WmT": np.ascontiguousarray(W_m.T),
        "WhhT": np.ascontiguousarray(W_hh.T),              # [VHS, 3*VHS]
        "WihT": np.ascontiguousarray(W_ih.T),              # [NVT, 3*VHS]
        "bg": np.asarray(inputs["b_g"], np.float32).reshape(VHS, 1),
        "bhh": np.ascontiguousarray(
            np.asarray(inputs["b_hh"], np.float32).reshape(3, VHS).T),  # [VHS,3]
        "bih": np.ascontiguousarray(
            np.asarray(inputs["b_ih"], np.float32).reshape(3, VHS).T),  # [VHS,3]
        "Wc1T": np.ascontiguousarray(W_c1.T),              # [VHS, CHS]
        "bc1": np.asarray(inputs["b_c1"], np.float32).reshape(CHS, 1),
        "Wc2T": np.ascontiguousarray(W_c2.T),              # [CHS, 1]
        "bc2": np.asarray(inputs["b_c2"], np.float32).reshape(1, 1),
    }

    in_maps = []
    for d in range(NDEV):
        xt = np.zeros((x.shape[1], M_tot), np.float32)
        for l in range(L):
            order_l = pp["order"][pp["cl0"][l]:pp["cl0"][l] + pp["cnt_l"][l]]
            nodes = order_l[d::NDEV]
            xt[:, own_off[l]:own_off[l] + len(nodes)] = x[nodes].T
        srcidx = np.concatenate(
            [idx_tile(pp["src0"][d]), idx_tile(pp["src1"][d])], axis=1)
        in_maps.append(dict(common, xt=xt, srcidx=srcidx))
    return in_maps, own_off, M_tot


def assemble_output(pp, results, L, N):
    m = pp["m"]
    own_off = np.zeros(L + 1, np.int64)
    own_off[1:] = np.cumsum(m)
    out = np.zeros((N, 1), np.float32)
    for d in range(NDEV):
        od = np.asarray(results[d]["out"]).reshape(-1)
        for l in range(L):
            order_l = pp["order"][pp["cl0"][l]:pp["cl0"][l] + pp["cnt_l"][l]]
            nodes = order_l[d::NDEV]
            out[nodes, 0] = od[own_off[l]:own_off[l] + len(nodes)]
    return out


# --------------------------------------------------------------------------
# device program
# --------------------------------------------------------------------------

def hoist_dma_waits(nc):
    """This toolchain's walrus codegen gives every TPB instruction a single
    embedded sync-wait slot; any instruction the Tile scheduler gave >1 wait
    fails codegen ("Too many sync wait commands").  Split the extra waits
    into standalone single-wait EventSemaphore ops on the same engine,
    placed immediately before the instruction (the sequencer stalls through
    them in order) — semantically equivalent, conservative ordering."""
    import concourse.mybir as mybir
    n = 0
    for f in nc.m.functions:
        for bb in f.blocks:
            insts = bb.instructions
            out = []
            changed = False
            for ins in insts:
                si = getattr(ins, "sync_info", None)
                eng = getattr(ins, "engine", None)
                if (si is not None and len(si.on_wait) > 1
                        and eng is not None
                        and not isinstance(ins, mybir.InstEventSemaphore)):
                    for j, wcond in enumerate(list(si.on_wait)[:-1]):
                        w = mybir.InstEventSemaphore(
                            name=f"{ins.name}-hw{j}", ins=[], outs=[])
                        w.engine = eng
                        w.sync_info = mybir.SyncInfo(
                            on_wait=[wcond], on_update=[])
                        out.append(w)
                        try:
                            nc.inst_map[w.name] = w
                        except Exception:
                            pass
                        n += 1
                    ins.sync_info = mybir.SyncInfo(
                        on_wait=[list(si.on_wait)[-1]],
                        on_update=list(si.on_update))
                    changed = True
                out.append(ins)
            if changed:
                bb.instructions = out
    return n


def build_program(pp, L, R, VHS, NVT, CHS, M_tot, reps=1):
    import concourse.bass as bass
    import concourse.mybir as mybir
    import concourse.tile as tile
    from concourse.masks import make_identity

    AF = mybir.ActivationFunctionType
    f32 = mybir.dt.float32
    i32 = mybir.dt.int32

    m = pp["m"]
    bs = pp["block_start"]
    n_lk = pp["n_lk"]
    lvl_off = pp["lvl_off"]
    maxdeg = pp["maxdeg"]
    E_pad_l = pp["E_pad_l"]
    step_off = pp["step_off"]
    TOT = pp["TOT"]
    N_pad = pp["N_pad"]
    zero_row = pp["zero_row"]
    own_off = np.zeros(L + 1, np.int64)
    own_off[1:] = np.cumsum(m)
    m_max = int(m.max())

    nc = bass.Bass(num_devices=NDEV)

    xt_ext = nc.declare_dram_parameter("xt", [NVT, M_tot], f32, isOutput=False)
    srcidx_ext = nc.declare_dram_parameter(
        "srcidx", [P, 2 * TOT // P], i32, isOutput=False)
    WgT_e = nc.declare_dram_parameter("WgT", [VHS, VHS], f32, isOutput=False)
    WmT_e = nc.declare_dram_parameter("WmT", [VHS, VHS], f32, isOutput=False)
    WhhT_e = nc.declare_dram_parameter("WhhT", [VHS, 3 * VHS], f32, isOutput=False)
    WihT_e = nc.declare_dram_parameter("WihT", [NVT, 3 * VHS], f32, isOutput=False)
    bg_e = nc.declare_dram_parameter("bg", [VHS, 1], f32, isOutput=False)
    bhh_e = nc.declare_dram_parameter("bhh", [VHS, 3], f32, isOutput=False)
    bih_e = nc.declare_dram_parameter("bih", [VHS, 3], f32, isOutput=False)
    Wc1T_e = nc.declare_dram_parameter("Wc1T", [VHS, CHS], f32, isOutput=False)
    bc1_e = nc.declare_dram_parameter("bc1", [CHS, 1], f32, isOutput=False)
    Wc2T_e = nc.declare_dram_parameter("Wc2T", [CHS, 1], f32, isOutput=False)
    bc2_e = nc.declare_dram_parameter("bc2", [1, 1], f32, isOutput=False)
    out_ext = nc.declare_dram_parameter("out", [1, M_tot], f32, isOutput=True)

    h_dram = nc.dram_tensor("h_dram", [N_pad + P, FW], f32, addr_space="Shared")
    stage_dram = nc.dram_tensor("stage_dram", [m_max, FW], f32)
    gi_dram = nc.dram_tensor("gi_dram", [3 * VHS, M_tot], f32)

    with tile.TileContext(nc) as tc:
        with (
            tc.tile_pool(name="const", bufs=1) as cp,
            tc.tile_pool(name="persist", bufs=1) as pe,
        ):
            # ---- constants
            ident = cp.tile([P, P], f32)
            make_identity(nc, ident[:])
            WgT = cp.tile([VHS, VHS], f32)
            nc.sync.dma_start(out=WgT[:], in_=WgT_e[:, :])
            WmT = cp.tile([VHS, VHS], f32)
            nc.sync.dma_start(out=WmT[:], in_=WmT_e[:, :])
            WhhT = cp.tile([VHS, 3 * VHS], f32)
            nc.sync.dma_start(out=WhhT[:], in_=WhhT_e[:, :])
            WihT = cp.tile([NVT, 3 * VHS], f32)
            nc.sync.dma_start(out=WihT[:], in_=WihT_e[:, :])
            bg = cp.tile([VHS, 1], f32)
            nc.sync.dma_start(out=bg[:], in_=bg_e[:, :])
            bhh = cp.tile([VHS, 3], f32)
            nc.sync.dma_start(out=bhh[:], in_=bhh_e[:, :])
            bih = cp.tile([VHS, 3], f32)
            nc.sync.dma_start(out=bih[:], in_=bih_e[:, :])
            Wc1T = cp.tile([VHS, CHS], f32)
            nc.sync.dma_start(out=Wc1T[:], in_=Wc1T_e[:, :])
            bc1 = cp.tile([CHS, 1], f32)
            nc.sync.dma_start(out=bc1[:], in_=bc1_e[:, :])
            Wc2T = cp.tile([CHS, 1], f32)
            nc.sync.dma_start(out=Wc2T[:], in_=Wc2T_e[:, :])
            bc2 = cp.tile([1, 1], f32)
            nc.sync.dma_start(out=bc2[:], in_=bc2_e[:, :])
            idx_sb = cp.tile([P, 2 * TOT // P], i32)
            nc.sync.dma_start(out=idx_sb[:], in_=srcidx_ext[:, :])

            # ---- persistent state
            hT_own = pe.tile([VHS, M_tot], f32)
            nc.any.memset(hT_own[:], 0.0)
            stage_sb = pe.tile([P, (m_max // P) * P], f32)
            nc.any.memset(stage_sb[:], 0.0)
            out_sb = pe.tile([1, M_tot], f32)
            # zero all padding rows of h_dram (incl. the dedicated zero row)
            nc.sync.dma_start(out=h_dram[N_pad:N_pad + P, :],
                              in_=stage_sb[:, :FW])

            # ---- gi = W_ih @ x^T + b_ih precompute -> gi_dram [3*VHS, M_tot]
            with (
                tc.tile_pool(name="init", bufs=2) as ip,
                tc.tile_pool(name="initps", bufs=1, space="PSUM") as ipp,
            ):
                xt_sb = ip.tile([NVT, M_tot], f32, tag="xt_sb", bufs=1)
                nc.sync.dma_start(out=xt_sb[:], in_=xt_ext[:, :])
                for g in range(3):
                    for c0 in range(0, M_tot, MMCHUNK):
                        c1 = min(c0 + MMCHUNK, M_tot)
                        pg = ipp.tile([VHS, MMCHUNK], f32, space="PSUM", tag="gi_ps")
                        nc.tensor.matmul(
                            out=pg[:, :c1 - c0],
                            lhsT=WihT[:, g * VHS:(g + 1) * VHS],
                            rhs=xt_sb[:, c0:c1], start=True, stop=True)
                        t = ip.tile([VHS, MMCHUNK], f32, tag="gi_sb")
                        nc.scalar.activation(
                            t[:, :c1 - c0], pg[:, :c1 - c0], AF.Identity,
                            bias=bih[:, g:g + 1])
                        nc.sync.dma_start(
                            out=gi_dram[g * VHS:(g + 1) * VHS, c0:c1],
                            in_=t[:, :c1 - c0])

            # ---- main recurrence
            with (
                tc.tile_pool(name="step", bufs=1) as sp,
                tc.tile_pool(name="gi", bufs=2) as gp,
                tc.tile_pool(name="tmp", bufs=1) as tp,
                tc.tile_pool(name="pt", bufs=2, space="PSUM") as ptp,
                tc.tile_pool(name="pmm", bufs=3, space="PSUM") as pmp,
                tc.tile_pool(name="pcls", bufs=2, space="PSUM") as pcp,
            ):
             for r in [rr for _ in range(reps) for rr in range(R)]:
                kind = 0 if r == 0 else 1
                for l in range(L):
                    ml = int(m[l])
                    Kl = int(E_pad_l[l]) // P
                    Epad = int(E_pad_l[l])
                    colbase = (kind * TOT + int(step_off[l])) // P

                    # prefetch gi slice for this step
                    gi_sb = gp.tile([VHS, 3 * m_max], f32, tag="gi_step")
                    for g in range(3):
                        nc.sync.dma_start(
                            out=gi_sb[:, g * ml:(g + 1) * ml],
                            in_=gi_dram[g * VHS:(g + 1) * VHS,
                                        int(own_off[l]):int(own_off[l]) + ml])

                    # 1. gather source rows (chunked indirect DMA)
                    gath = sp.tile([P, Kl * P], f32, tag="gath")
                    for c0 in range(0, Kl, GATHER_COLS):
                        kc = min(GATHER_COLS, Kl - c0)
                        nc.gpsimd.indirect_dma_start(
                            out=gath[:, c0 * P:(c0 + kc) * P],
                            out_offset=None,
                            in_=h_dram[:, :],
                            in_offset=bass.IndirectOffsetOnAxis(
                                ap=idx_sb[:, colbase + c0:colbase + c0 + kc],
                                axis=0))

                    # 2. transpose to feature-major
                    hsT = sp.tile([P, Kl * P], f32, tag="hsT")
                    for j in range(Kl):
                        pt = ptp.tile([P, P], f32, space="PSUM", tag="tr_ps")
                        nc.tensor.transpose(
                            out=pt[:], in_=gath[:, j * P:(j + 1) * P],
                            identity=ident[:])
                        nc.any.tensor_copy(
                            out=hsT[:, j * P:(j + 1) * P], in_=pt[:])

                    # 3. gn = sigmoid(Wg h + bg) * (Wm h) per edge slot
                    gnT = sp.tile([VHS, Kl * P], f32, tag="gnT")
                    for c0 in range(0, Epad, MMCHUNK):
                        c1 = min(c0 + MMCHUNK, Epad)
                        pg = pmp.tile([VHS, MMCHUNK], f32, space="PSUM", tag="mm_ps")
                        nc.tensor.matmul(out=pg[:, :c1 - c0], lhsT=WgT[:],
                                         rhs=hsT[:VHS, c0:c1],
                                         start=True, stop=True)
                        pm = pmp.tile([VHS, MMCHUNK], f32, space="PSUM", tag="mm_ps")
                        nc.tensor.matmul(out=pm[:, :c1 - c0], lhsT=WmT[:],
                                         rhs=hsT[:VHS, c0:c1],
                                         start=True, stop=True)
                        sg = tp.tile([VHS, MMCHUNK], f32, tag="sig")
                        nc.scalar.activation(sg[:, :c1 - c0], pg[:, :c1 - c0],
                                             AF.Sigmoid, bias=bg[:, :])
                        nc.any.tensor_mul(gnT[:, c0:c1], sg[:, :c1 - c0],
                                          pm[:, :c1 - c0])

                    # 4. segment sum via level adds
                    msgT = sp.tile([VHS, m_max], f32, tag="msgT")
                    nc.any.tensor_copy(out=msgT[:, :ml], in_=gnT[:, :ml])
                    for k in range(1, maxdeg):
                        nk = int(n_lk[l, k])
                        if nk == 0:
                            break
                        o = int(lvl_off[l, k])
                        nc.any.tensor_add(msgT[:, :nk], msgT[:, :nk],
                                          gnT[:, o:o + nk])

                    # 5. GRU (+ classifier on last round) per column chunk
                    for c0 in range(0, ml, MMCHUNK):
                        c1 = min(c0 + MMCHUNK, ml)
                        w = c1 - c0
                        ph = [None] * 3
                        for g in range(3):
                            ph[g] = pmp.tile([VHS, MMCHUNK], f32, space="PSUM",
                                             tag="mm_ps", name=f"gh{g}")
                            nc.tensor.matmul(
                                out=ph[g][:, :w],
                                lhsT=WhhT[:, g * VHS:(g + 1) * VHS],
                                rhs=msgT[:, c0:c1], start=True, stop=True)
                        gi_r = gi_sb[:, 0 * ml + c0:0 * ml + c1]
                        gi_z = gi_sb[:, 1 * ml + c0:1 * ml + c1]
                        gi_n = gi_sb[:, 2 * ml + c0:2 * ml + c1]
                        t_r = tp.tile([VHS, MMCHUNK], f32, tag="t_r")
                        nc.any.tensor_add(t_r[:, :w], gi_r, ph[0][:, :w])
                        rg = tp.tile([VHS, MMCHUNK], f32, tag="rg")
                        nc.scalar.activation(rg[:, :w], t_r[:, :w], AF.Sigmoid,
                                             bias=bhh[:, 0:1])
                        t_z = tp.tile([VHS, MMCHUNK], f32, tag="t_z")
                        nc.any.tensor_add(t_z[:, :w], gi_z, ph[1][:, :w])
                        zg = tp.tile([VHS, MMCHUNK], f32, tag="zg")
                        nc.scalar.activation(zg[:, :w], t_z[:, :w], AF.Sigmoid,
                                             bias=bhh[:, 1:2])
                        hn = tp.tile([VHS, MMCHUNK], f32, tag="hn")
                        nc.scalar.activation(hn[:, :w], ph[2][:, :w], AF.Identity,
                                             bias=bhh[:, 2:3])
                        rhn = tp.tile([VHS, MMCHUNK], f32, tag="rhn")
                        nc.any.tensor_mul(rhn[:, :w], rg[:, :w], hn[:, :w])
                        tc_ = tp.tile([VHS, MMCHUNK], f32, tag="tc_")
                        nc.any.tensor_add(tc_[:, :w], rhn[:, :w], gi_n)
                        cg = tp.tile([VHS, MMCHUNK], f32, tag="cg")
                        nc.scalar.activation(cg[:, :w], tc_[:, :w], AF.Tanh)
                        hp = hT_own[:, int(own_off[l]) + c0:int(own_off[l]) + c1]
                        d1 = tp.tile([VHS, MMCHUNK], f32, tag="d1")
                        nc.any.tensor_sub(d1[:, :w], hp, cg[:, :w])
                        d2 = tp.tile([VHS, MMCHUNK], f32, tag="d2")
                        nc.any.tensor_mul(d2[:, :w], zg[:, :w], d1[:, :w])
                        nc.any.tensor_add(hp, cg[:, :w], d2[:, :w])

                        if r == R - 1:
                            pc1 = pcp.tile([CHS, MMCHUNK], f32, space="PSUM",
                                           tag="cls_ps")
                            nc.tensor.matmul(out=pc1[:, :w], lhsT=Wc1T[:],
                                             rhs=hp, start=True, stop=True)
                            c1t = tp.tile([CHS, MMCHUNK], f32, tag="c1t")
                            nc.scalar.activation(c1t[:, :w], pc1[:, :w], AF.Relu,
                                                 bias=bc1[:, :])
                            pc2 = pcp.tile([1, MMCHUNK], f32, space="PSUM",
                                           tag="cls_ps")
                            nc.tensor.matmul(out=pc2[:, :w], lhsT=Wc2T[:],
                                             rhs=c1t[:, :w], start=True, stop=True)
                            nc.scalar.activation(
                                out_sb[:, int(own_off[l]) + c0:
                                       int(own_off[l]) + c1],
                                pc2[:, :w], AF.Sigmoid, bias=bc2[:, :])

                    # 6. h_new -> row-major staging -> AllGather into h_dram
                    for j in range(ml // P):
                        pt = ptp.tile([P, P], f32, space="PSUM", tag="tr_ps")
                        nc.tensor.transpose(
                            out=pt[:, :VHS],
                            in_=hT_own[:, int(own_off[l]) + j * P:
                                       int(own_off[l]) + (j + 1) * P],
                            identity=ident[:VHS, :VHS])
                        nc.any.tensor_copy(
                            out=stage_sb[:, j * P:j * P + VHS],
                            in_=pt[:, :VHS])
                    nc.sync.dma_start(
                        out=stage_dram[0:ml, :].rearrange(
                            "(j p) f -> p j f", p=P),
                        in_=stage_sb[:, :ml // P * P].rearrange(
                            "p (j f) -> p j f", f=P))
                    nc.gpsimd.collective_compute(
                        "AllGather",
                        bass.mybir.AluOpType.bypass,
                        replica_groups=[list(range(NDEV))],
                        ins=[stage_dram[0:ml, :]],
                        outs=[h_dram[int(bs[l]):int(bs[l]) + NDEV * ml, :]],
                    )
             nc.sync.dma_start(out=out_ext[:, :], in_=out_sb[:])
    hoist_dma_waits(nc)
    return nc


# --------------------------------------------------------------------------
# entry point
# --------------------------------------------------------------------------

def kernel(**inputs):
    import sys
    if "/opt/trn_rl_repo" not in sys.path:
        sys.path.insert(0, "/opt/trn_rl_repo")
    from concourse.bass_utils import run_bass_kernel_spmd

    x = np.asarray(inputs["x"], np.float32)
    node_layer = np.asarray(inputs["node_layer"])
    edge_src = np.asarray(inputs["edge_src"])
    edge_dst = np.asarray(inputs["edge_dst"])
    L = int(inputs["num_layers"])
    R = int(inputs["nrounds"])
    N = x.shape[0]
    NVT = x.shape[1]
    VHS = np.asarray(inputs["W_g"]).shape[0]
    CHS = np.asarray(inputs["W_c1"]).shape[0]

    pp = preprocess(node_layer, edge_src, edge_dst, L)
    in_maps, own_off, M_tot = build_core_inputs(pp, inputs, L)
    nc = build_program(pp, L, R, VHS, NVT, CHS, M_tot)
    res = run_bass_kernel_spmd(nc, in_maps, core_ids=list(range(NDEV)))
    return assemble_output(pp, res.results, L, N)
